# revision 10
# baseline (speedup 1.0000x reference)
"""Averaged-key circular-convolutional attention on 8 trn2 NeuronCores.

Per (b,h): out_h = Circ(attn/N) @ V_h with attn = softmax(Q_h . mean_n(K_h) * SCALE).
Key identities used:
  - K_avg = mean_n(x) @ Wk.T  (mean is linear -> no K projection needed)
  - circulant matmul done block-wise: out_blk[i1] = sum_m M_m @ V_blk[(i1+m)%16]
    where M_m[i0,j0] = c[(m*128 + j0 - i0) mod N], built by one overlapping-window
    DMA from a DRAM scratch F2 = [c; c] (free index flipped so all AP steps are +1;
    the row flip is undone by doing the PE transpose against the exchange matrix J).
  - softmax max-subtraction dropped: |z*SCALE/N| << 1 for this input distribution.
Sharding: 24 (b,h) pairs -> 3 per core (core c: b=c//4, heads 3*(c%4)..+2). Each core
emits partial_out.T = Wp_sub @ OHT (768, 2048) fp16; host sums the 4 partials per b.
"""
import numpy as np
import ml_dtypes

N = 2048
C = 768
HEADS = 12
D = 64
SCALE = D ** -0.5
NB = 16      # number of 128-blocks in N
BLK = 128

_CACHE = {}


def _build_nc(repeat=1):
    import concourse.bass as bass
    import concourse.tile as tile
    from concourse import bacc, mybir

    f32 = mybir.dt.float32
    f16 = mybir.dt.float16
    bf16 = mybir.dt.bfloat16
    AX = mybir.AxisListType.X
    EXP = mybir.ActivationFunctionType.Exp
    IDN = mybir.ActivationFunctionType.Identity

    nc = bacc.Bacc("TRN2", target_bir_lowering=False, debug=False, num_devices=8)

    xt = nc.dram_tensor("xt", [C, N], bf16, kind="ExternalInput")          # x[b].T
    wv = nc.dram_tensor("wv", [C, 192], bf16, kind="ExternalInput")        # Wv_h.T
    wqn = nc.dram_tensor("wqn", [192, C], bf16, kind="ExternalInput")      # Wq_h natural
    wk = nc.dram_tensor("wk", [C, 192], bf16, kind="ExternalInput")        # Wk_h.T
    wp = nc.dram_tensor("wp", [192, C], bf16, kind="ExternalInput")        # Wp[:,cols].T
    jm = nc.dram_tensor("jm", [BLK, BLK], bf16, kind="ExternalInput")      # exchange
    im = nc.dram_tensor("im", [BLK, BLK], bf16, kind="ExternalInput")      # identity
    bpv = nc.dram_tensor("bpv", [C], f32, kind="ExternalInput")            # bias or 0
    out = nc.dram_tensor("out", [C, N], f16, kind="ExternalOutput")        # partial.T

    f2a = nc.dram_tensor("f2a", [3, 2 * N], bf16)                          # scratch [c;c] per head

    SCALE_P = SCALE / N  # K_avg is an unnormalized sum -> fold 1/N here

    with tile.TileContext(nc) as tc:
        with (
            tc.tile_pool(name="big", bufs=1) as big,       # long-lived tensors
            tc.tile_pool(name="work", bufs=3) as work,     # cycling small tiles
            tc.tile_pool(name="ps", bufs=2, space="PSUM") as ps,
            tc.tile_pool(name="tps", bufs=2, space="PSUM") as tps,
            tc.tile_pool(name="pv", bufs=2, space="PSUM") as pv,
        ):
          for _rep in range(repeat):
            # ---------- loads (xt chunked so proj starts early) ----------
            wv_sb = big.tile([BLK, 6 * 192], bf16, tag="wv")
            nc.sync.dma_start(wv_sb[:, :], bass.AP(wv, 0, [[192, BLK], [BLK * 192, 6], [1, 192]]))
            wqnA_sb = big.tile([BLK, C], bf16, tag="wqnA")
            nc.sync.dma_start(wqnA_sb[:, :], wqn[0:BLK, :])
            wqnB_sb = big.tile([D, C], bf16, tag="wqnB")
            nc.sync.dma_start(wqnB_sb[:, :], wqn[BLK:192, :])
            xt_sb = big.tile([BLK, 6 * N], bf16, tag="xt")
            for cc in range(6):
                nc.sync.dma_start(xt_sb[:, cc * N:(cc + 1) * N],
                                  bass.AP(xt, cc * BLK * N, [[N, BLK], [1, N]]))
            wk_sb = big.tile([BLK, 6 * 192], bf16, tag="wk")
            nc.sync.dma_start(wk_sb[:, :], bass.AP(wk, 0, [[192, BLK], [BLK * 192, 6], [1, 192]]))
            wpA_sb = big.tile([BLK, C], bf16, tag="wpA")
            nc.sync.dma_start(wpA_sb[:, :], wp[0:BLK, :])
            wpB_sb = big.tile([D, C], bf16, tag="wpB")
            nc.sync.dma_start(wpB_sb[:, :], wp[BLK:192, :])
            j_sb = big.tile([BLK, BLK], bf16, tag="jm")
            nc.sync.dma_start(j_sb[:, :], jm[:, :])
            i_sb = big.tile([BLK, BLK], bf16, tag="im")
            nc.sync.dma_start(i_sb[:, :], im[:, :])
            bp_sb = big.tile([BLK, 6], f32, tag="bp")
            nc.sync.dma_start(bp_sb[:, :], bass.AP(bpv, 0, [[1, BLK], [BLK, 6]]))

            # ---------- x_sum (split DVE/ACT) and K_avg (needed before z) ----------
            xm_sb = work.tile([BLK, 6], f32, tag="xm")
            xdump = work.tile([BLK, N], bf16, tag="xdump")
            for cc in range(6):
                if cc % 2 == 0:
                    nc.vector.reduce_sum(xm_sb[:, cc:cc + 1], xt_sb[:, cc * N:(cc + 1) * N], axis=AX)
                else:
                    nc.scalar.activation(xdump[:, :], xt_sb[:, cc * N:(cc + 1) * N],
                                         IDN, accum_out=xm_sb[:, cc:cc + 1])
            xmb_sb = work.tile([BLK, 6], bf16, tag="xmb")
            nc.vector.tensor_copy(xmb_sb[:, :], xm_sb[:, :])
            kps = ps.tile([BLK, 1], f32, tag="ps")
            for cc in range(6):
                nc.tensor.matmul(kps[:, :], wk_sb[:, cc * 192: cc * 192 + BLK],
                                 xmb_sb[:, cc:cc + 1], start=(cc == 0), stop=(cc == 5))
            kavgA = work.tile([BLK, 1], bf16, tag="kavgA")
            nc.vector.tensor_copy(kavgA[:, :], kps[:, :])
            kps2 = ps.tile([D, 1], f32, tag="ps")
            for cc in range(6):
                nc.tensor.matmul(kps2[:, :], wk_sb[:, cc * 192 + BLK: cc * 192 + 192],
                                 xmb_sb[:, cc:cc + 1], start=(cc == 0), stop=(cc == 5))
            kavgB = work.tile([D, 1], bf16, tag="kavgB")
            nc.vector.tensor_copy(kavgB[:, :], kps2[:, :])

            # ---------- w_z = Wq_h.T @ K_avg_h (Q projection folded away) ----------
            wz_sb = work.tile([BLK, 6 * 3], bf16, tag="wz")
            for h in range(3):
                if h == 0:
                    lwq, kav = wqnA_sb[0:D, :], kavgA[0:D, :]
                elif h == 1:
                    lwq, kav = wqnA_sb[D:BLK, :], kavgA[D:BLK, :]
                else:
                    lwq, kav = wqnB_sb[0:D, :], kavgB[0:D, :]
                pwz = pv.tile([BLK, 6], f32, tag="pv", name=f"pwz{h}")
                for cc in range(6):
                    nc.tensor.matmul(pwz[:, cc:cc + 1], lwq[:, cc * BLK:(cc + 1) * BLK],
                                     kav, start=True, stop=True)
                dstz = wz_sb[:, :].rearrange("p (cc h) -> p cc h", h=3)
                nc.vector.tensor_copy(dstz[:, :, h:h + 1],
                                      pwz[:, :].rearrange("p (cc o) -> p cc o", o=1))

            # ---------- z for all 3 heads at once: Z3 = W_z.T @ x.T (3, 2048) ----------
            toep = [big.tile([BLK, N], bf16, tag=f"toep{h}", name=f"toep{h}") for h in range(3)]
            zps = [ps.tile([3, 1024], f32, tag="ps", name=f"zps{i}") for i in range(2)]
            for half in range(2):
                for cc in range(6):
                    for q in range(2):
                        nc.tensor.matmul(
                            zps[half][:, q * 512:(q + 1) * 512],
                            wz_sb[:, cc * 3:(cc + 1) * 3],
                            xt_sb[:, cc * N + half * 1024 + q * 512: cc * N + half * 1024 + (q + 1) * 512],
                            start=(cc == 0), stop=(cc == 5), skip_group_check=True)
            # batched softmax over 3 heads (no max-subtraction: |z*SCALE_P| << 1)
            ex = work.tile([3, N], f32, tag="ex")
            ssum = work.tile([3, 2], f32, tag="ssum")
            for half in range(2):
                nc.scalar.activation(
                    ex[:, half * 1024:(half + 1) * 1024], zps[half][:, :],
                    EXP, bias=0.0, scale=SCALE_P,
                    accum_out=ssum[:, half:half + 1])
            stot = work.tile([3, 1], f32, tag="stot")
            nc.vector.reduce_sum(stot[:, :], ssum[:, :], axis=AX)
            rin = work.tile([3, 1], f32, tag="rin")
            nc.vector.reciprocal(rin[:, :], stot[:, :])
            nc.scalar.mul(rin[:, :], rin[:, :], 1.0 / N)
            c2w = work.tile([3, 2 * N], bf16, tag="c2w")
            nc.vector.tensor_scalar_mul(c2w[:, 0:N], ex[:, :], rin[:, :])
            nc.scalar.activation(c2w[:, N:2 * N], ex[:, :], IDN, scale=rin[:, :])
            nc.sync.dma_start(f2a[:, :], c2w[:, :])
            for h in range(3):
                nc.sync.dma_start(toep[h][:, :],
                                  bass.AP(f2a, h * 2 * N + N - 127, [[1, BLK], [1, N]]))

            # ---------- V projection directly in conv layout (xt blocks stationary) ----------
            # vt0[j0, i1*64+dd] = V_h0[i1*128+j0, dd];  vtp[j0, i1*128 + (h-1)*64 + dd]
            # both padded with 15 extra blocks so conv rhs never wraps
            vt0 = big.tile([BLK, (NB + 15) * D], bf16, tag="vt0")
            vtp = big.tile([BLK, (NB + 15) * BLK], bf16, tag="vtp")
            for blk in range(NB):
                pvv = pv.tile([BLK, 192], f32, tag="pv", name=f"pvv{blk}")
                for cc in range(6):
                    nc.tensor.matmul(pvv[:, :],
                                     xt_sb[:, cc * N + blk * BLK: cc * N + (blk + 1) * BLK],
                                     wv_sb[:, cc * 192:(cc + 1) * 192],
                                     start=(cc == 0), stop=(cc == 5))
                nc.vector.tensor_copy(vt0[:, blk * D:(blk + 1) * D], pvv[:, 0:D])
                if blk % 2 == 0:
                    nc.vector.tensor_copy(vtp[:, blk * BLK:(blk + 1) * BLK], pvv[:, D:192])
                else:
                    nc.scalar.activation(vtp[:, blk * BLK:(blk + 1) * BLK], pvv[:, D:192], IDN)
            # wrap padding
            nc.vector.tensor_copy(vt0[:, NB * D:(NB + 15) * D], vt0[:, 0:15 * D])
            nc.scalar.activation(vtp[:, NB * BLK:(NB + 15) * BLK], vtp[:, 0:15 * BLK], IDN)

            # ---------- convs (dense PE, no wrap splits) ----------
            oh01 = big.tile([BLK, N], bf16, tag="oh01")   # interleaved h0|h1 per block
            oh2 = big.tile([BLK, NB * D], bf16, tag="oh2")
            rhs3 = vtp[:, :].rearrange("p (g x) -> p g x", x=BLK)
            for h in range(3):
                poh = ps.tile([BLK, 1024], f32, tag="ps")
                for m in range(NB):
                    lw = toep[h][:, m * BLK:(m + 1) * BLK]
                    for q in range(2):
                        a = q * 8          # psum block start
                        v = m + q * 8      # vt block start (padded, no wrap)
                        if h == 0:
                            rhs = vt0[:, v * D:(v + 8) * D]
                        else:
                            rhs = rhs3[:, v:v + 8, (h - 1) * D:h * D]
                        nc.tensor.matmul(
                            poh[:, a * D:(a + 8) * D], lw, rhs,
                            start=(m == 0), stop=(m == NB - 1 and q == 1),
                            skip_group_check=True)
                # copy conv result (rows flipped) to bf16 SBUF
                if h == 2:
                    nc.vector.tensor_copy(oh2[:, :], poh[:, :])
                else:
                    dst = oh01[:, :].rearrange("p (g x) -> p g x", x=BLK)
                    src = poh[:, :].rearrange("p (g x) -> p g x", x=D)
                    if h == 0:
                        nc.vector.tensor_copy(dst[:, :, 0:D], src)
                    else:
                        nc.scalar.activation(dst[:, :, D:BLK], src, IDN)

            # ---------- OH transposes (J undoes the row flip) -> OHT ----------
            ohTA = big.tile([BLK, N], bf16, tag="ohTA")
            ohTB = big.tile([D, N], bf16, tag="ohTB")
            for blk in range(NB):
                ta = tps.tile([BLK, BLK], bf16, tag="tp")
                nc.tensor.transpose(ta[:, :], oh01[:, blk * BLK:(blk + 1) * BLK], j_sb[:, :])
                if blk % 2 == 0:
                    nc.vector.tensor_copy(ohTA[:, blk * BLK:(blk + 1) * BLK], ta[:, :])
                else:
                    nc.scalar.activation(ohTA[:, blk * BLK:(blk + 1) * BLK], ta[:, :], IDN)
                tb = tps.tile([D, BLK], bf16, tag="tp")
                nc.tensor.transpose(tb[:, :], oh2[:, blk * D:(blk + 1) * D], j_sb[:, :])
                nc.vector.tensor_copy(ohTB[:, blk * BLK:(blk + 1) * BLK], tb[:, :])

            # ---------- final projection: partial.T = Wp_sub @ OHT + bp ----------
            for cc in range(6):
                for half in range(2):
                    pf = ps.tile([BLK, 1024], f32, tag="ps")
                    for q in range(2):
                        sl = slice(half * 1024 + q * 512, half * 1024 + (q + 1) * 512)
                        nc.tensor.matmul(pf[:, q * 512:(q + 1) * 512],
                                         wpA_sb[:, cc * BLK:(cc + 1) * BLK], ohTA[:, sl],
                                         start=True, stop=False, skip_group_check=True)
                        nc.tensor.matmul(pf[:, q * 512:(q + 1) * 512],
                                         wpB_sb[:, cc * BLK:(cc + 1) * BLK], ohTB[:, sl],
                                         start=False, stop=True, skip_group_check=True)
                    fo = work.tile([BLK, 1024], f16, tag="fo")
                    if half == 0:
                        nc.vector.tensor_scalar_add(fo[:, :], pf[:, :], bp_sb[:, cc:cc + 1])
                    else:
                        nc.scalar.activation(fo[:, :], pf[:, :], IDN,
                                             bias=bp_sb[:, cc:cc + 1], scale=1.0)
                    nc.sync.dma_start(out[cc * BLK:(cc + 1) * BLK, half * 1024:(half + 1) * 1024],
                                      fo[:, :])
    nc.finalize()
    return nc


def _get_nc(repeat=1):
    key = ("nc", repeat)
    if key not in _CACHE:
        _CACHE[key] = _build_nc(repeat)
    return _CACHE[key]


def make_in_maps(x, Wq, Wk, Wv, Wp, bp):
    bf = ml_dtypes.bfloat16
    jm = np.eye(BLK)[::-1].astype(bf).copy()
    im = np.eye(BLK).astype(bf).copy()
    in_maps = []
    for core in range(8):
        b, g = core // 4, core % 4
        rows = slice(g * 192, (g + 1) * 192)
        in_maps.append({
            "xt": np.ascontiguousarray(x[b].T).astype(bf),
            "wv": np.ascontiguousarray(Wv[rows].T).astype(bf),
            "wqn": np.ascontiguousarray(Wq[rows]).astype(bf),
            "wk": np.ascontiguousarray(Wk[rows].T).astype(bf),
            "wp": np.ascontiguousarray(Wp[:, rows].T).astype(bf),
            "jm": jm,
            "im": im,
            "bpv": (bp if g == 0 else np.zeros_like(bp)).astype(np.float32),
        })
    return in_maps


def gather(results):
    outs = []
    for b in range(2):
        tot = results[4 * b]["out"].astype(np.float32)
        for g in range(1, 4):
            tot = tot + results[4 * b + g]["out"].astype(np.float32)
        outs.append(tot.T)
    return np.stack(outs, axis=0)


def run_spmd(in_maps, trace=False, **kw):
    from concourse.bass_utils import run_bass_kernel_spmd
    return run_bass_kernel_spmd(_get_nc(), in_maps, core_ids=list(range(8)),
                                trace=trace, **kw)


def kernel(x, Wq, Wk, Wv, Wp, bp):
    x = np.asarray(x, dtype=np.float32)
    res = run_spmd(make_in_maps(np.asarray(x, np.float32), np.asarray(Wq, np.float32),
                                np.asarray(Wk, np.float32), np.asarray(Wv, np.float32),
                                np.asarray(Wp, np.float32), np.asarray(bp, np.float32)))
    return gather(res.results)
